# revision 1
# baseline (speedup 1.0000x reference)
"""Dcls1d (Gaussian-parameterized dilated conv1d) Trainium2 Bass kernel.

Math (reference):
    W   = weight * sign                               (O, I, C)
    Pc  = P[0] + KD//2 ; S = |SIG[0]| + 0.27          (O, I, C)
    X_d = exp(-0.5 * ((d - Pc)/S)^2)                  d = 0..KD-1
    K   = sum_c X_d * W / (sum_d' X_d' + 1e-7)        (O, I, KD)
    out = conv1d(x, K, VALID)                         (B, O, L-KD+1)

Distribution over 8 NeuronCores:
  - kernel construction: out-channel-sharded (32 out-channels per core)
  - AllGather of the small kernel in matmul-ready lhsT layout
  - conv: batch-sharded (4 batches per core), bf16 accumulating PE matmuls

The work is split into two out-channel halves: half B's construction
(DVE/ACT engines) overlaps half A's conv (PE engine), with one AllGather
per half in between.

Construction layout per core & half: partitions p = i mod 128,
free = (j, c), j = ih*16 + o_lo over 32 tiles of 128 (o,i)-pairs.
Per-d Gaussian argument is one fused scalar_tensor_tensor
m = (Pc - d) * R (sign irrelevant, m is squared), and the Gaussian itself
is one ScalarE op: erf'(m/sqrt2) = (2/sqrt(pi)) * exp(-m^2/2), with the
2/sqrt(pi) folded into the normalization epsilon. X is stored bf16;
Z = sum_d X_d is a bf16 pairwise tree (2x-mode adds).
"""

import os

import numpy as np

import concourse.bass as bass
import concourse.mybir as mybir
import concourse.tile as tile
from concourse import bacc
from concourse.bass_utils import run_bass_kernel_spmd

F32 = mybir.dt.float32
BF16 = mybir.dt.bfloat16
AF = mybir.ActivationFunctionType
ALU = mybir.AluOpType

B, OC, IC, L = 32, 256, 256, 1024
KC, KD = 26, 25
NC = 8
O_SH = OC // NC          # 32 out-channels per core
NIB = IC // 128          # 2 i-blocks
NH = 2                   # out-channel halves (pipeline stages)
O_H = O_SH // NH         # 16 out-channels per core per half
NT = O_H * NIB           # 32 construction tiles per half
FB = NT * KC             # 832 free width per half
B_SH = B // NC           # 4 batches per core
TO = L - KD + 1          # 1000 output positions
TC = 500                 # conv t-chunk (PSUM bank = 512 fp32 max)
NTC = TO // TC           # 2


def build_module():
    nc = bacc.Bacc("TRN2", num_devices=NC)

    p_in = nc.dram_tensor("p_in", [128, NH * FB], F32, kind="ExternalInput")
    sig_in = nc.dram_tensor("sig_in", [128, NH * FB], F32, kind="ExternalInput")
    w_in = nc.dram_tensor("w_in", [128, NH * FB], F32, kind="ExternalInput")
    sgn_in = nc.dram_tensor("sgn_in", [128, NH * FB], F32, kind="ExternalInput")
    x_in = nc.dram_tensor("x_in", [B_SH, NIB, 128, L], F32, kind="ExternalInput")
    out_t = nc.dram_tensor("out", [B_SH, OC, TO], F32, kind="ExternalOutput")

    kshard = [
        nc.dram_tensor(f"kshard{h}", [KD, NIB, 128, O_H], BF16)
        for h in range(NH)
    ]
    kgath = [
        nc.dram_tensor(
            f"kgath{h}", [NC, KD, NIB, 128, O_H], BF16, addr_space="Shared"
        )
        for h in range(NH)
    ]

    use_derf = os.environ.get("DCLS_SIM_EXP", "0") != "1"
    c_gauss = 1.1283791670955126 if use_derf else 1.0

    with tile.TileContext(nc) as tc:
        with tc.tile_pool(name="smalls", bufs=1) as smalls, \
             tc.tile_pool(name="hp", bufs=1) as hp, \
             tc.tile_pool(name="dtmp", bufs=2) as dtmp, \
             tc.tile_pool(name="kw", bufs=2) as kw, \
             tc.tile_pool(name="xp", bufs=1) as xp, \
             tc.tile_pool(name="ps", bufs=4, space="PSUM") as ps, \
             tc.tile_pool(name="obp", bufs=3) as obp:
            # ---- head: load inputs, full-width small tensors ----
            p_sb = smalls.tile([128, NH * FB], F32)
            sig_sb = smalls.tile([128, NH * FB], F32)
            w_sb = smalls.tile([128, NH * FB], F32)
            sgn_sb = smalls.tile([128, NH * FB], F32)
            nc.sync.dma_start(p_sb[:], p_in[:])
            nc.sync.dma_start(sig_sb[:], sig_in[:])
            nc.sync.dma_start(w_sb[:], w_in[:])
            nc.sync.dma_start(sgn_sb[:], sgn_in[:])

            x_sb = {}
            for b in range(B_SH):
                for ih in range(NIB):
                    t = xp.tile([128, L], BF16, tag=f"x{b}_{ih}")
                    # casting DMA (f32 -> bf16) on the software DGE
                    nc.gpsimd.dma_start(t[:], x_in[b, ih, :, :])
                    x_sb[(b, ih)] = t

            pc_sb = p_sb
            nc.vector.tensor_scalar_add(pc_sb[:], p_sb[:], float(KD // 2))
            nc.scalar.activation(sig_sb[:], sig_sb[:], AF.Abs)
            nc.vector.tensor_scalar_add(sig_sb[:], sig_sb[:], 0.27)
            rscr = smalls.tile([128, NH * FB], F32)
            r_sb = sig_sb
            nc.vector.reciprocal_approx_accurate(r_sb[:], sig_sb[:], rscr[:])
            wp_sb = w_sb
            nc.vector.tensor_mul(wp_sb[:], w_sb[:], sgn_sb[:])

            lhsT = {}
            for h in range(NH):
                sl = slice(h * FB, (h + 1) * FB)
                pc_h, r_h, wp_h = pc_sb[:, sl], r_sb[:, sl], wp_sb[:, sl]

                # ---- Gaussian: X'_d = c * exp(-0.5*((Pc-d)*R)^2), bf16 ----
                x_all = hp.tile([128, KD * FB], BF16, tag="xall")
                for d in range(KD):
                    m = dtmp.tile([128, FB], F32, tag="m")
                    nc.vector.scalar_tensor_tensor(
                        m[:], pc_h, float(d), r_h,
                        op0=ALU.subtract, op1=ALU.mult,
                    )
                    if use_derf:
                        nc.scalar.activation(
                            x_all[:, d * FB:(d + 1) * FB], m[:],
                            AF.Derivative_Erf, scale=0.7071067811865476,
                        )
                    else:
                        nc.scalar.activation(m[:], m[:], AF.Square)
                        nc.scalar.activation(
                            x_all[:, d * FB:(d + 1) * FB], m[:], AF.Exp,
                            scale=-0.5,
                        )

                # ---- Z = sum_d X_d: bf16 4-way groups + tree ----
                zbuf = hp.tile([128, 8 * FB], BF16, tag="zbuf")
                zs = [zbuf[:, i * FB:(i + 1) * FB] for i in range(8)]
                xs = [x_all[:, d * FB:(d + 1) * FB] for d in range(KD)]
                z_sb = hp.tile([128, FB], F32, tag="z")
                with nc.allow_low_precision("bf16 partial sums"):
                    for g in range(6):
                        nc.vector.tensor_add(zs[6], xs[4 * g], xs[4 * g + 1])
                        nc.vector.tensor_add(zs[7], xs[4 * g + 2], xs[4 * g + 3])
                        nc.vector.tensor_add(zs[g], zs[6], zs[7])
                    nc.vector.tensor_add(zs[6], zs[0], zs[1])
                    nc.vector.tensor_add(zs[7], zs[2], zs[3])
                    nc.vector.tensor_add(zs[0], zs[4], zs[5])
                    nc.vector.tensor_add(zs[1], zs[6], zs[7])
                    nc.vector.tensor_add(zs[2], zs[1], zs[0])
                    nc.vector.tensor_add(z_sb[:], zs[2], xs[KD - 1])

                # ---- Wn16 = bf16(Wp / (Z + c*1e-7)) ----
                nc.vector.tensor_scalar_add(z_sb[:], z_sb[:], c_gauss * 1e-7)
                nc.vector.reciprocal_approx_accurate(
                    z_sb[:], z_sb[:], rscr[:, 0:FB]
                )
                wn16 = hp.tile([128, FB], BF16, tag="wn16")
                with nc.allow_low_precision("bf16 conv weights"):
                    nc.vector.tensor_mul(wn16[:], wp_h, z_sb[:])

                    # ---- K[p,(d,ih,ol)] = sum_c X'_d * Wn ----
                    k_sb = hp.tile([128, KD * NT], BF16, tag="ksb")
                    for d in range(KD):
                        y = dtmp.tile([128, FB], BF16, tag="y")
                        nc.vector.tensor_mul(
                            y[:], x_all[:, d * FB:(d + 1) * FB], wn16[:]
                        )
                        y3 = y.rearrange("p (j c) -> p j c", c=KC)
                        nc.vector.reduce_sum(
                            k_sb[:, d * NT:(d + 1) * NT], y3,
                            axis=mybir.AxisListType.X,
                        )

                # ---- store shard + all-gather this half ----
                ksb_v = k_sb.rearrange(
                    "p (d ih ol) -> p d ih ol", ih=NIB, ol=O_H
                )
                kout_v = kshard[h][:].rearrange("d ih p ol -> p d ih ol")
                nc.sync.dma_start(kout_v, ksb_v)
                nc.gpsimd.collective_compute(
                    "AllGather",
                    ALU.bypass,
                    replica_groups=[list(range(NC))],
                    ins=[kshard[h][:]],
                    outs=[kgath[h][:]],
                )

                # ---- conv for this half's 128 out-channels ----
                for d in range(KD):
                    for ih in range(NIB):
                        t = kw.tile([128, NC * O_H], BF16, tag=f"k{d}_{ih}")
                        src = kgath[h][:, d, ih, :, :].rearrange(
                            "core p ol -> p core ol"
                        )
                        nc.sync.dma_start(
                            t[:].rearrange("p (core ol) -> p core ol", core=NC),
                            src,
                        )
                        lhsT[(d, ih)] = t

                # out rows for this half: o = 32*core + 16*h + ol
                out_v = out_t[:].rearrange(
                    "b (core half ol) t -> b half core ol t", core=NC, half=NH
                )
                for b in range(B_SH):
                    for tck in range(NTC):
                        acc = ps.tile([128, TC], F32, tag="acc")
                        n = 0
                        for ih in range(NIB):
                            for d in range(KD):
                                nc.tensor.matmul(
                                    acc[:],
                                    lhsT[(d, ih)][:],
                                    x_sb[(b, ih)][:, tck * TC + d:
                                                  tck * TC + d + TC],
                                    start=(n == 0),
                                    stop=(n == NIB * KD - 1),
                                )
                                n += 1
                        o_sb = obp.tile([128, TC], F32, tag="osb")
                        nc.scalar.copy(o_sb[:], acc[:])
                        dst = out_v[b, h, :, :, tck * TC:(tck + 1) * TC]
                        nc.sync.dma_start(dst, o_sb[:])

    nc.compile()
    return nc


def make_in_maps(x, weight, sign, P, SIG):
    """Slice/pack full inputs into per-core input maps (pure layout work)."""
    x = np.ascontiguousarray(x, dtype=np.float32)
    in_maps = []
    for c in range(NC):
        osl = slice(O_SH * c, O_SH * c + O_SH)

        def pack(a):
            # (O_SH, IC, KC) -> [p = i mod 128, (half, j = ih*16+ol, c)]
            a = np.asarray(a, dtype=np.float32).reshape(NH, O_H, NIB, 128, KC)
            a = a.transpose(0, 2, 1, 3, 4)          # (half, ih, ol, p, c)
            a = a.reshape(NH * NT, 128, KC)
            return np.ascontiguousarray(
                a.transpose(1, 0, 2).reshape(128, NH * NT * KC)
            )

        in_maps.append({
            "p_in": pack(P[0][osl]),
            "sig_in": pack(SIG[0][osl]),
            "w_in": pack(weight[osl]),
            "sgn_in": pack(sign[osl]),
            "x_in": np.ascontiguousarray(
                x[B_SH * c: B_SH * c + B_SH].reshape(B_SH, NIB, 128, L)
            ),
        })
    return in_maps


_CACHED = {}


def kernel(x, weight, sign, P, SIG, trace=False):
    if "nc" not in _CACHED:
        _CACHED["nc"] = build_module()
    nc = _CACHED["nc"]
    in_maps = make_in_maps(x, weight, sign, P, SIG)
    res = run_bass_kernel_spmd(
        nc, in_maps, core_ids=list(range(NC)), trace=trace,
    )
    out = np.concatenate([r["out"] for r in res.results], axis=0)
    if trace:
        _CACHED["last_result"] = res
    return out



# revision 2
# speedup vs baseline: 1.4861x; 1.4861x over previous
"""Dcls1d (Gaussian-parameterized dilated conv1d) Trainium2 Bass kernel.

Math (reference):
    W   = weight * sign                               (O, I, C)
    Pc  = P[0] + KD//2 ; S = |SIG[0]| + 0.27          (O, I, C)
    X_d = exp(-0.5 * ((d - Pc)/S)^2)                  d = 0..KD-1
    K   = sum_c X_d * W / (sum_d' X_d' + 1e-7)        (O, I, KD)
    out = conv1d(x, K, VALID)                         (B, O, L-KD+1)

Distribution over 8 NeuronCores:
  - kernel construction: out-channel-sharded (32 out-channels per core)
  - AllGather of the small kernel in matmul-ready lhsT layout
  - conv: batch-sharded (4 batches per core), bf16 PE matmuls

v2 pipeline: construction is split into 4 chunks (out-channel half x
i-block), each AllGathered independently so the conv can start after the
first chunk (~25% of construction) instead of after half of it. The conv
is weight-stationary: for each (d, ih) weight tile, 8 matmuls (4 batches
x 2 t-chunks) stream into 8 PSUM banks, keeping the PE busy enough that
the HAM clock gate stays at full rate. A dummy AllGather at t=0 absorbs
the collective-stream setup/barrier skew.

Construction per chunk: partitions p = i mod 128, free = (o_lo, c).
m_d = d*R - Pc*R is one fp16 scalar_tensor_tensor (2x DVE mode);
X'_d = erf'(m/sqrt2) = (2/sqrt(pi)) exp(-m^2/2) is one ScalarE op with
the 2/sqrt(pi) folded into the normalization epsilon. Z = sum_d X_d is
a wide strided bf16 tree; the c-contraction is an in-place bf16 multiply
per d plus one wide reduce.
"""

import numpy as np

import concourse.bass as bass
import concourse.mybir as mybir
import concourse.tile as tile
from concourse import bacc
from concourse.bass_utils import run_bass_kernel_spmd

F32 = mybir.dt.float32
BF16 = mybir.dt.bfloat16
FP16 = mybir.dt.float16
AF = mybir.ActivationFunctionType
ALU = mybir.AluOpType

B, OC, IC, L = 32, 256, 256, 1024
KC, KD = 26, 25
NC = 8
O_SH = OC // NC          # 32 out-channels per core
NIB = IC // 128          # 2 i-blocks
NH = 2                   # out-channel halves
O_H = O_SH // NH         # 16 out-channels per core per half
NCH = NH * NIB           # 4 construction chunks (half, ih)
FW = O_H * KC            # 416 free width per chunk
B_SH = B // NC           # 4 batches per core
TO = L - KD + 1          # 1000 output positions
TC = 500                 # conv t-chunk (PSUM bank = 512 fp32 max)
NTC = TO // TC           # 2

C_GAUSS = 1.1283791670955126  # 2/sqrt(pi), from derf(x) = c*exp(-x^2)


def build_module():
    nc = bacc.Bacc("TRN2", num_devices=NC)

    p_in = nc.dram_tensor("p_in", [128, NCH * FW], F32, kind="ExternalInput")
    sig_in = nc.dram_tensor("sig_in", [128, NCH * FW], F32, kind="ExternalInput")
    w_in = nc.dram_tensor("w_in", [128, NCH * FW], F32, kind="ExternalInput")
    sgn_in = nc.dram_tensor("sgn_in", [128, NCH * FW], F32, kind="ExternalInput")
    x_in = nc.dram_tensor("x_in", [B_SH, NIB, 128, L], F32, kind="ExternalInput")
    out_t = nc.dram_tensor("out", [B_SH, OC, TO], F32, kind="ExternalOutput")

    kshard = [
        nc.dram_tensor(f"kshard{c}", [KD, 128, O_H], BF16) for c in range(NCH)
    ]
    kgath = [
        nc.dram_tensor(
            f"kgath{c}", [NC, KD, 128, O_H], BF16, addr_space="Shared"
        )
        for c in range(NCH)
    ]
    dum_in = nc.dram_tensor("dum_in", [1, 64], BF16)
    dum_out = nc.dram_tensor("dum_out", [NC, 64], BF16, addr_space="Shared")

    with tile.TileContext(nc) as tc:
        with tc.tile_pool(name="smalls", bufs=1) as smalls, \
             tc.tile_pool(name="csm", bufs=2) as csm, \
             tc.tile_pool(name="hp", bufs=2) as hp, \
             tc.tile_pool(name="dtmp", bufs=2) as dtmp, \
             tc.tile_pool(name="kw", bufs=4) as kw, \
             tc.tile_pool(name="xp", bufs=1) as xp, \
             tc.tile_pool(name="ps", bufs=1, space="PSUM") as ps, \
             tc.tile_pool(name="obp", bufs=3) as obp:
            # ---- head: dummy AG (warms CC stream), input loads ----
            nc.gpsimd.collective_compute(
                "AllGather", ALU.bypass,
                replica_groups=[list(range(NC))],
                ins=[dum_in[:]], outs=[dum_out[:]],
            )
            p_sb = smalls.tile([128, NCH * FW], F32)
            sig_sb = smalls.tile([128, NCH * FW], F32)
            w_sb = smalls.tile([128, NCH * FW], F32)
            sgn_sb = smalls.tile([128, NCH * FW], F32)
            nc.sync.dma_start(p_sb[:], p_in[:])
            nc.sync.dma_start(sig_sb[:], sig_in[:])
            nc.sync.dma_start(w_sb[:], w_in[:])
            nc.sync.dma_start(sgn_sb[:], sgn_in[:])

            x_sb = {}
            for b in range(B_SH):
                for ih in range(NIB):
                    t = xp.tile([128, L], BF16, tag=f"x{b}_{ih}")
                    # casting DMA (f32 -> bf16) on the software DGE
                    nc.gpsimd.dma_start(t[:], x_in[b, ih, :, :])
                    x_sb[(b, ih)] = t

            # ---- construction chunks ----
            lhsT = {}
            for ch in range(NCH):
                sl = slice(ch * FW, (ch + 1) * FW)
                pc, sg = p_sb[:, sl], sig_sb[:, sl]
                wp, sn = w_sb[:, sl], sgn_sb[:, sl]

                r32 = csm.tile([128, FW], F32, tag="r32")
                r16 = csm.tile([128, FW], FP16, tag="r16")
                u16 = csm.tile([128, FW], FP16, tag="u16")
                wn16 = csm.tile([128, FW], BF16, tag="wn16")
                z32 = csm.tile([128, FW], F32, tag="z32")
                zr32 = csm.tile([128, FW], F32, tag="zr32")

                nc.vector.tensor_scalar_add(pc, pc, float(KD // 2))
                nc.scalar.activation(sg, sg, AF.Abs)
                nc.vector.tensor_scalar_add(sg, sg, 0.27)
                nc.vector.reciprocal_approx_fast(r32[:], sg)
                nc.scalar.copy(r16[:], r32[:])
                # u = -Pc*R
                nc.vector.scalar_tensor_tensor(
                    u16[:], pc, -1.0, r32[:], op0=ALU.mult, op1=ALU.mult,
                )
                nc.vector.tensor_mul(wp, wp, sn)

                # ---- Gaussian: X'_d = c * exp(-0.5*((d - Pc)*R)^2), bf16 ----
                x_all = hp.tile([128, KD * FW], BF16, tag="xall")
                for d in range(KD):
                    m = dtmp.tile([128, FW], FP16, tag="m")
                    # m = d*R - Pc*R  (all fp16: 2x DVE mode)
                    nc.vector.scalar_tensor_tensor(
                        m[:], r16[:], float(d), u16[:],
                        op0=ALU.mult, op1=ALU.add,
                    )
                    nc.scalar.activation(
                        x_all[:, d * FW:(d + 1) * FW], m[:],
                        AF.Derivative_Erf, scale=0.7071067811865476,
                    )

                # ---- Z = sum_d X_d: wide strided bf16 tree ----
                zA = hp.tile([128, 6 * FW], BF16, tag="zA")
                zB = hp.tile([128, 6 * FW], BF16, tag="zB")
                xg = x_all[:, 0:24 * FW].rearrange(
                    "p (g four f) -> p g four f", four=4, f=FW
                )
                zA3 = zA[:].rearrange("p (g f) -> p g f", f=FW)
                zB3 = zB[:].rearrange("p (g f) -> p g f", f=FW)
                with nc.allow_low_precision("bf16 partial sums"):
                    nc.vector.tensor_add(zA3, xg[:, :, 0, :], xg[:, :, 1, :])
                    nc.vector.tensor_add(zB3, xg[:, :, 2, :], xg[:, :, 3, :])
                    # zA[g] = sum of 4 slabs
                    nc.vector.tensor_add(zA[:], zA[:], zB[:])
                    zg = zA[:].rearrange("p (t two f) -> p t two f", two=2, f=FW)
                    zD3 = zB[:, 0:3 * FW].rearrange("p (t f) -> p t f", f=FW)
                    nc.vector.tensor_add(zD3, zg[:, :, 0, :], zg[:, :, 1, :])
                    nc.vector.tensor_add(
                        zB[:, 3 * FW:4 * FW], zB[:, 0:FW], zB[:, FW:2 * FW]
                    )
                    nc.vector.tensor_add(
                        zB[:, 4 * FW:5 * FW],
                        zB[:, 3 * FW:4 * FW], zB[:, 2 * FW:3 * FW],
                    )
                    # final add picks up the 25th slab, f32 out
                    nc.vector.tensor_add(
                        z32[:], zB[:, 4 * FW:5 * FW],
                        x_all[:, 24 * FW:25 * FW],
                    )

                # ---- Wn = bf16(Wp / (Z + c*1e-7)) ----
                nc.vector.tensor_scalar_add(z32[:], z32[:], C_GAUSS * 1e-7)
                nc.vector.reciprocal_approx_fast(zr32[:], z32[:])
                with nc.allow_low_precision("bf16 conv weights"):
                    nc.vector.tensor_mul(wn16[:], wp, zr32[:])

                    # ---- K[p,(d,j)] = sum_c X'_d * Wn: in-place muls + reduce
                    for d in range(KD):
                        xs = x_all[:, d * FW:(d + 1) * FW]
                        nc.vector.tensor_mul(xs, xs, wn16[:])
                    k_sb = hp.tile([128, KD * O_H], BF16, tag="ksb")
                    y3 = x_all[:].rearrange("p (dj c) -> p dj c", c=KC)
                    nc.vector.reduce_sum(
                        k_sb[:], y3, axis=mybir.AxisListType.X,
                    )

                # ---- store shard + all-gather this chunk ----
                ksb_v = k_sb[:].rearrange("p (d j) -> p d j", j=O_H)
                kout_v = kshard[ch][:].rearrange("d p j -> p d j")
                nc.gpsimd.dma_start(kout_v, ksb_v)
                nc.gpsimd.collective_compute(
                    "AllGather", ALU.bypass,
                    replica_groups=[list(range(NC))],
                    ins=[kshard[ch][:]], outs=[kgath[ch][:]],
                )
                for d in range(KD):
                    t = kw.tile([128, NC * O_H], BF16, tag=f"k{d}")
                    src = kgath[ch][:, d, :, :].rearrange("core p j -> p core j")
                    nc.sync.dma_start(
                        t[:].rearrange("p (core j) -> p core j", core=NC), src,
                    )
                    lhsT[(ch, d)] = t

            # ---- conv: weight-stationary, 8 PSUM banks ----
            out_v = out_t[:].rearrange(
                "b (core half ol) t -> b half core ol t", core=NC, half=NH
            )
            for h in range(NH):
                accs = {}
                for b in range(B_SH):
                    for tck in range(NTC):
                        accs[(b, tck)] = ps.tile(
                            [128, TC], F32, tag=f"acc{b}_{tck}",
                            name=f"acc_{h}_{b}_{tck}",
                        )
                for ih in range(NIB):
                    ch = h * NIB + ih
                    for d in range(KD):
                        lt = lhsT[(ch, d)]
                        for b in range(B_SH):
                            for tck in range(NTC):
                                nc.tensor.matmul(
                                    accs[(b, tck)][:], lt[:],
                                    x_sb[(b, ih)][:, tck * TC + d:
                                                  tck * TC + d + TC],
                                    start=(ih == 0 and d == 0),
                                    stop=(ih == NIB - 1 and d == KD - 1),
                                )
                for b in range(B_SH):
                    for tck in range(NTC):
                        o_sb = obp.tile(
                            [128, TC], F32, tag="osb", name=f"o_{h}_{b}_{tck}"
                        )
                        nc.scalar.copy(o_sb[:], accs[(b, tck)][:])
                        dst = out_v[b, h, :, :, tck * TC:(tck + 1) * TC]
                        nc.sync.dma_start(dst, o_sb[:])

    nc.compile()
    return nc


def make_in_maps(x, weight, sign, P, SIG):
    """Slice/pack full inputs into per-core input maps (pure layout work)."""
    x = np.ascontiguousarray(x, dtype=np.float32)
    in_maps = []
    for c in range(NC):
        osl = slice(O_SH * c, O_SH * c + O_SH)

        def pack(a):
            # (O_SH, IC, KC) -> [p = i mod 128, (half, ih, o_lo, c)]
            a = np.asarray(a, dtype=np.float32).reshape(NH, O_H, NIB, 128, KC)
            a = a.transpose(0, 2, 1, 3, 4)          # (half, ih, ol, p, c)
            a = a.reshape(NCH * O_H, 128, KC)
            return np.ascontiguousarray(
                a.transpose(1, 0, 2).reshape(128, NCH * O_H * KC)
            )

        in_maps.append({
            "p_in": pack(P[0][osl]),
            "sig_in": pack(SIG[0][osl]),
            "w_in": pack(weight[osl]),
            "sgn_in": pack(sign[osl]),
            "x_in": np.ascontiguousarray(
                x[B_SH * c: B_SH * c + B_SH].reshape(B_SH, NIB, 128, L)
            ),
        })
    return in_maps


_CACHED = {}


def kernel(x, weight, sign, P, SIG, trace=False):
    if "nc" not in _CACHED:
        _CACHED["nc"] = build_module()
    nc = _CACHED["nc"]
    in_maps = make_in_maps(x, weight, sign, P, SIG)
    res = run_bass_kernel_spmd(
        nc, in_maps, core_ids=list(range(NC)), trace=trace,
    )
    out = np.concatenate([r["out"] for r in res.results], axis=0)
    if trace:
        _CACHED["last_result"] = res
    return out
